# revision 21
# baseline (speedup 1.0000x reference)
"""Distributed MHA kernel for 8 Trainium2 NeuronCores.

Sharding: core i handles batch b = i//2, head-group g = i%2 (8 of 16 heads).
Data parallel on B, tensor parallel on H: column-parallel QKV, row-parallel
output projection with the partial sums reduced on the host during gather.

Math (per core, heads h in its group, E=1024, H=16, d=64, N=1024):
  QT[hd, n] = sum_e Wq[hd, e] x[n, e] + bq[hd]        (transposed layout)
  KT[hd, n] = likewise
  V[n, hd]  = sum_e x[n, e] Wv[hd, e]                  (bv folded on host)
  energyT_h[k, q] = sum_d KT_h[d, k] QT_h[d, q]
  expT_h = exp(energyT_h)          (no max-subtract; |energy| < ~50 is safe)
  outT_h_aug[0:64, q] = sum_k V_h[k, d] expT_h[k, q]   (ones column in V
  outT_h_aug[64, q]   = sum_k expT_h[k, q] = den_h[q]   gives den for free)
  norm_h[d, q] = outT_h[d, q] * (1/den_h[q])
  y_part[q, e] = sum_{h,d} norm_h[d, q] Wo[e, 64h+d]
Host: out[b] = (y_part[2b] + y_part[2b+1])/32 + (bo + Wo @ bv / 32)
(att rows sum to 1/sqrt(E)=1/32 exactly, so bv contributes Wo@bv/32.)

v2 schedule: stage B units (p, qs) are software-pipelined — unit u's
att@v matmuls are interleaved into unit u+1's energy matmuls so the PE
never waits on the exp stream; exp activations are batched over 2-bank
PSUM groups ([128,1024]) to amortize ACT fixed overhead; DMAs ordered by
first use (biases first, x/wq/wk interleaved pair-sliced, then wv; wo
last).  x/Wq/Wk/Wv ship as fp16 (halves input DMA; ~0.05% element error
is negligible next to the bf16 exp quantization).  The 1/32 softmax
descale lives in the host gather.
PSUM budget: proj 2 banks + energy groups 4 + att@v accumulators 2 = 8.
CoreSim cost-model estimate: ~135 us/core (baseline was ~185 us).
"""

import numpy as np

import concourse.bass as bass
import concourse.tile as tile
from concourse import mybir
from concourse.bass_utils import run_bass_kernel_spmd

E = 1024
N = 1024
B = 4
NC = 8
EH = 512          # head dims per core (8 heads x 64)
D = 64
BF16 = mybir.dt.bfloat16
F32 = mybir.dt.float32
AX = mybir.AluOpType
F32R = mybir.dt.float32r

F16 = mybir.dt.float16

# Input precision: fp16 (10-bit mantissa) halves DMA bytes at ~0.05%
# element error -- negligible next to the bf16 exp-weight quantization.
# (fp8e4m3 for the V/output projections was measured at 3-4e-2 output
# error -- over the 2e-2 budget -- so everything PE-facing stays fp16.)
# The 1/32 softmax descale lives in the host gather.
XDT = F16
WQKDT = F16
WVDT = F16
WODT = F32R


def split_drain_waits(nc):
    """Walrus in this toolchain rejects instructions carrying more than one
    sem wait; move extra waits onto injected same-engine NOPs placed right
    before the instruction (same engine queue = program order preserved).

    Iterate over snapshots: take_nop appends to a live instruction list, so
    iterating the live list would re-encounter (duplicate) the stolen NOPs."""
    snaps = {name: list(w.bb.instructions) for name, w in nc.bb_map.items()}

    def take_nop(engine):
        nop = nc.engines[engine].nop(nofuse=True).ins
        for bname, bw in nc.bb_map.items():
            lst = bw.bb.instructions
            if lst and lst[-1].name == nop.name:
                bw.bb.instructions = lst[:-1]
                break
        return nop

    for name, w in nc.bb_map.items():
        new_insts = []
        changed = False
        for ins in snaps[name]:
            si = ins.sync_info
            if si is not None and si.on_wait and len(si.on_wait) > 1:
                waits = list(si.on_wait)
                for wt in waits[:-1]:
                    nop = take_nop(ins.engine)
                    nop.sync_info = mybir.SyncInfo(on_wait=[wt], on_update=[])
                    new_insts.append(nop)
                si.on_wait = waits[-1:]
                ins.sync_info = si
                changed = True
            new_insts.append(ins)
        if changed:
            w.bb.instructions = new_insts


def _emit(nc: bass.Bass, tc: tile.TileContext, ctx):
    xT = nc.declare_dram_parameter("xT", [E, N], XDT, isOutput=False)
    wqT = nc.declare_dram_parameter("wqT", [E, EH], WQKDT, isOutput=False)
    wkT = nc.declare_dram_parameter("wkT", [E, EH], WQKDT, isOutput=False)
    wvT = nc.declare_dram_parameter("wvT", [E, EH], WVDT, isOutput=False)
    woT = nc.declare_dram_parameter("woT", [EH, E], WODT, isOutput=False)
    bqd = nc.declare_dram_parameter("bq", [128, 4], F32, isOutput=False)
    bkd = nc.declare_dram_parameter("bk", [128, 4], F32, isOutput=False)
    y = nc.declare_dram_parameter("y", [N, E], F32, isOutput=True)

    persist = ctx.enter_context(tc.tile_pool(name="persist", bufs=1))
    etp = ctx.enter_context(tc.tile_pool(name="etp", bufs=10))
    nrm = ctx.enter_context(tc.tile_pool(name="nrm", bufs=2))
    ytrans = ctx.enter_context(tc.tile_pool(name="ytrans", bufs=2))
    dram = ctx.enter_context(tc.tile_pool(name="dram", bufs=2, space="DRAM"))
    # PSUM: proj 2 banks + energy groups 2x2 banks + po 2 banks = 8 total
    pproj = ctx.enter_context(tc.tile_pool(name="pproj", bufs=2, space="PSUM"))
    pegr = ctx.enter_context(tc.tile_pool(name="pegr", bufs=2, space="PSUM"))
    ppo = ctx.enter_context(tc.tile_pool(name="ppo", bufs=1, space="PSUM"))

    # ---- input DMAs, ordered by first use ----
    bq_t = persist.tile([128, 4], F32, tag="bqt", name="bq_t")
    nc.sync.dma_start(out=bq_t, in_=bqd[:, :])
    bk_t = persist.tile([128, 4], F32, tag="bkt", name="bk_t")
    nc.sync.dma_start(out=bk_t, in_=bkd[:, :])
    bq_sb = [bq_t[:, m:m + 1] for m in range(4)]
    bk_sb = [bk_t[:, m:m + 1] for m in range(4)]
    # wq/wk tiles are allocated whole ([128, 512]) but loaded in pair-column
    # slices so pair 0's slices (needed by qk(0)) land first.
    xt, wq, wk, wv = [], [], [], []
    for e in range(8):
        t = persist.tile([128, N], XDT, tag=f"xt{e}", name=f"xt{e}")
        nc.sync.dma_start(out=t, in_=xT[e * 128:(e + 1) * 128, :])
        xt.append(t)
        for lst, src, nm in ((wq, wqT, "wq"), (wk, wkT, "wk")):
            t = persist.tile([128, EH], WQKDT, tag=f"{nm}{e}", name=f"{nm}{e}")
            nc.sync.dma_start(out=t[:, 0:128],
                              in_=src[e * 128:(e + 1) * 128, 0:128])
            lst.append(t)
    for e in range(8):
        t = persist.tile([128, EH], WVDT, tag=f"wv{e}", name=f"wv{e}")
        nc.sync.dma_start(out=t, in_=wvT[e * 128:(e + 1) * 128, :])
        wv.append(t)
    for p in range(1, 4):
        for e in range(8):
            for lst, src in ((wq, wqT), (wk, wkT)):
                nc.sync.dma_start(
                    out=lst[e][:, p * 128:(p + 1) * 128],
                    in_=src[e * 128:(e + 1) * 128, p * 128:(p + 1) * 128])
    wo = []
    for p in range(4):
        t = persist.tile([128, E], WODT, tag=f"wo{p}", name=f"wo{p}")
        nc.sync.dma_start(out=t, in_=woT[p * 128:(p + 1) * 128, :])
        wo.append(t)

    qt = [persist.tile([128, N], F32R, tag=f"qt{p}", name=f"qt{p}")
          for p in range(4)]
    kt = [persist.tile([128, N], F32R, tag=f"kt{p}", name=f"kt{p}")
          for p in range(4)]
    vt = [persist.tile([128, 8, 65], BF16, tag=f"v{n}", name=f"v{n}")
          for n in range(8)]
    pack = [[persist.tile([128, 512], F32R, tag=f"pk{p}_{qs}",
                          name=f"pk{p}_{qs}") for qs in range(2)]
            for p in range(4)]

    # ---- emission helpers ----
    def emit_qk_half(p, which):
        """One of q/k for pair p: both 512-col halves."""
        w_t, b_t, dst = ((wq, bq_sb, qt) if which == "q" else (wk, bk_sb, kt))
        for half in range(2):
            ps = pproj.tile([128, 512], F32, tag="ps", name="psqk")
            for e in range(8):
                nc.tensor.matmul(
                    out=ps, lhsT=(w_t[e][:, p * 128:(p + 1) * 128]),
                    rhs=(xt[e][:, half * 512:(half + 1) * 512]),
                    start=(e == 0), stop=(e == 7))
            nc.vector.tensor_scalar_add(
                dst[p][:, half * 512:(half + 1) * 512], ps, b_t[p])

    def emit_v(n):
        ps = pproj.tile([128, 512], F32, tag="ps", name="psv")
        for e in range(8):
            nc.tensor.matmul(
                out=ps, lhsT=(xt[e][:, n * 128:(n + 1) * 128]), rhs=(wv[e]),
                start=(e == 0), stop=(e == 7))
        nc.vector.memset(vt[n][:, :, 64:65], 1.0)
        nc.vector.tensor_copy(
            vt[n][:, :, 0:64], ps.rearrange("p (h d) -> p h d", h=8))

    def emit_energy(p, qs, k):
        """Energy for both head-halves of pair p, k-tile k, into one 2-bank
        PSUM group; exp over the whole group in one ACT instruction."""
        eg = pegr.tile([128, 1024], F32, tag="eg", name="eg")
        for ab in range(2):
            nc.tensor.matmul(
                out=eg[:, ab * 512:(ab + 1) * 512],
                lhsT=(kt[p][ab * 64:(ab + 1) * 64, k * 128:(k + 1) * 128]),
                rhs=(qt[p][ab * 64:(ab + 1) * 64, qs * 512:(qs + 1) * 512]),
                start=True, stop=True)
        et = etp.tile([128, 1024], BF16, tag="et", name="et")
        nc.scalar.activation(out=et, in_=eg,
                             func=mybir.ActivationFunctionType.Exp)
        return et

    def emit_attv(p, qs, k, et, po):
        for ab in range(2):
            nc.tensor.matmul(
                out=po[ab][0:65, :], lhsT=vt[k][:, 2 * p + ab, :],
                rhs=et[:, ab * 512:(ab + 1) * 512],
                start=(k == 0), stop=(k == 7))

    def emit_norm(p, qs, po):
        """Normalize: pack = att_out / den, den broadcast over 64 partitions
        via a DRAM round-trip (stride-0 partition AP on the read back);
        po is evacuated to SBUF first so the PSUM bank frees fast."""
        for ab in range(2):
            pos = nrm.tile([65, 512], F32, tag=f"pos{ab}", name=f"pos{ab}")
            nc.vector.tensor_copy(pos, po[ab][0:65, :])
            s = nrm.tile([1, 512], F32, tag=f"s{ab}", name=f"s{ab}")
            nc.vector.reciprocal(out=s, in_=pos[64:65, :])
            sdr = dram.tile([1, 512], F32, tag=f"sdr{ab}", name=f"sdr{ab}")
            nc.sync.dma_start(out=sdr, in_=s)
            srep = nrm.tile([64, 512], F32, tag=f"srep{ab}", name=f"srep{ab}")
            nc.sync.dma_start(
                out=srep,
                in_=bass.AP(tensor=sdr.tensor, offset=sdr.offset,
                            ap=[[0, 64]] + list(sdr.ap[1:])))
            if ab == 0:
                nc.vector.scalar_tensor_tensor(
                    out=pack[p][qs][0:64, :], in0=pos[0:64, :],
                    scalar=1.0, in1=srep, op0=AX.mult, op1=AX.mult)
            else:
                tmp = nrm.tile([64, 512], F32R, tag="otB", name="otB")
                nc.vector.scalar_tensor_tensor(
                    out=tmp, in0=pos[0:64, :],
                    scalar=1.0, in1=srep, op0=AX.mult, op1=AX.mult)
                nc.sync.dma_start(out=pack[p][qs][64:128, :], in_=tmp)

    def emit_c(qt_i, es):
        qs, qq = qt_i // 4, qt_i % 4
        ps = pproj.tile([128, 512], F32, tag="ps", name="psy")
        for p in range(4):
            nc.tensor.matmul(
                out=ps, lhsT=(pack[p][qs][:, qq * 128:(qq + 1) * 128]),
                rhs=(wo[p][:, es * 512:(es + 1) * 512]),
                start=(p == 0), stop=(p == 3))
        ys = ytrans.tile([128, 512], F32, tag="ysb", name="ysb")
        nc.vector.tensor_copy(ys, ps)
        nc.sync.dma_start(
            out=y[qt_i * 128:(qt_i + 1) * 128, es * 512:(es + 1) * 512],
            in_=ys)

    # ---- pipelined emission ----
    emit_qk_half(0, "q")
    emit_qk_half(0, "k")

    units = [(p, qs) for p in range(4) for qs in range(2)]
    prev = None          # (p, qs, ets, po) of the previous unit
    for ui, (p, qs) in enumerate(units):
        po = [ppo.tile([128, 512], F32, tag=f"po{ab}", name=f"po{ab}")
              for ab in range(2)]
        ets = []
        for k in range(8):
            ets.append(emit_energy(p, qs, k))
            if prev is not None:
                emit_attv(prev[0], prev[1], k, prev[2][k], prev[3])
        if prev is not None:
            emit_norm(prev[0], prev[1], prev[3])
        # V projection right after unit 0's energies: ACT starts ASAP and
        # vt is ready before unit 1 (which carries unit 0's att@v matmuls)
        if ui == 0:
            for n in range(8):
                emit_v(n)
        # feed the PE queue with next pair's projections while ACT works
        if qs == 1 and p < 3:
            emit_qk_half(p + 1, "q")
            emit_qk_half(p + 1, "k")
        # overlap the first half of stage C with the last unit's exp stream
        if (p, qs) == (3, 1):
            for qt_i in range(4):
                for es in range(2):
                    emit_c(qt_i, es)
        prev = (p, qs, ets, po)

    for k in range(8):
        emit_attv(prev[0], prev[1], k, prev[2][k], prev[3])
    emit_norm(prev[0], prev[1], prev[3])
    for qt_i in range(4, 8):
        for es in range(2):
            emit_c(qt_i, es)


def build():
    from contextlib import ExitStack
    nc = bass.Bass()
    with tile.TileContext(nc) as tc:
        with ExitStack() as ctx:
            _emit(nc, tc, ctx)
    split_drain_waits(nc)
    return nc


def make_in_maps(x, Wq, bq, Wk, bk, Wv, bv, Wo, bo):
    in_maps = []
    for i in range(NC):
        b, g = i // 2, i % 2
        sl = slice(g * EH, (g + 1) * EH)
        in_maps.append({
            "xT": np.ascontiguousarray(x[b].T).astype(np.float16),
            "wqT": np.ascontiguousarray(Wq[sl, :].T).astype(np.float16),
            "wkT": np.ascontiguousarray(Wk[sl, :].T).astype(np.float16),
            "wvT": np.ascontiguousarray(Wv[sl, :].T).astype(np.float16),
            "woT": np.ascontiguousarray(Wo[:, sl].T),
            "bq": np.ascontiguousarray(bq[sl].reshape(4, 128).T).astype(np.float32),
            "bk": np.ascontiguousarray(bk[sl].reshape(4, 128).T).astype(np.float32),
        })
    return in_maps


def gather(results, Wv_b, Wo, bv, bo):
    host_bias = (bo + Wo @ bv / 32.0).astype(np.float32)
    out = np.empty((B, N, E), np.float32)
    for b in range(B):
        out[b] = (results[2 * b]["y"] + results[2 * b + 1]["y"]) / 32.0 \
            + host_bias
    return out


def kernel(x, Wq, bq, Wk, bk, Wv, bv, Wo, bo):
    x, Wq, bq, Wk, bk, Wv, bv, Wo, bo = [
        np.asarray(a, np.float32) for a in (x, Wq, bq, Wk, bk, Wv, bv, Wo, bo)]
    nc = build()
    in_maps = make_in_maps(x, Wq, bq, Wk, bk, Wv, bv, Wo, bo)
    res = run_bass_kernel_spmd(nc, in_maps, list(range(NC)))
    return gather(res.results, Wv, Wo, bv, bo)


if __name__ == "__main__":
    import reference
    inputs = {k: np.asarray(v) for k, v in reference.setup_inputs().items()}
    out = kernel(**inputs)
    exp = np.asarray(reference.reference(**inputs))
    rel = np.abs(out - exp).max() / np.abs(exp).max()
    print("Relative error:", rel)
